# revision 1
# baseline (speedup 1.0000x reference)
"""ConvBERT attention block (SeparableConv1D key + dynamic conv) on 8 TRN2 NeuronCores.

Sharding: data-parallel over batch (B=8 -> 1 sample per core), weights replicated.

Per-core dataflow (all activations in [C, L] layout, channels on partitions):
  xT [768, 2048] bf16 (host-pretransposed)
  conv: dw_out[c, l] = sum_k xT[c, l+k-4] * dw[c, k]   (PE diag-matmuls + DVE MACs)
  q|co = W_qco^T @ xT        (TensorE, K=C contraction, out [o, l])
  key  = pw^T @ dw_out       (TensorE)
  attn = key * q             (DVE)
  kernT_pre = W_ck^T @ attn  -> [108, L] logits, exp on ACT (softmax w/o max: logits are tiny)
  sums = ones_block @ expT   -> [12, L]; recip on DVE
  einsum: out[c, l] = (sum_k co[c, l+k-4] * expT[h(c)*9+k, l]) * recip[h(c), l]
          (DVE: 1 windowed 9-tap mult + in-place tree adds + norm mult)
  out [768, 2048] bf16 -> host transposes back to [L, C] f32.
"""

import os
import sys

for _p in ("/opt/trn_rl_repo", "/root/.axon_site/_ro/trn_rl_repo"):
    if os.path.isdir(_p) and _p not in sys.path:
        sys.path.append(_p)

import ml_dtypes
import numpy as np

import concourse.bass as bass
import concourse.mybir as mybir
import concourse.tile as tile
from concourse import bacc
from concourse.bass_utils import run_bass_kernel_spmd
from concourse.masks import make_identity

BF16 = mybir.dt.bfloat16
F32 = mybir.dt.float32

H, D, K = 12, 64, 9
C = H * D  # 768
L = 2048
B = 8
PAD = (K - 1) // 2  # 4
P = 128
NCT = C // P  # 6 channel tiles
LC = 512  # l-chunk (one PSUM bank of f32)
NLC = L // LC  # 4
HK = H * K  # 108
PE_CONV_CT = 4  # ctiles [0, PE_CONV_CT) do depthwise conv on TensorE, rest on DVE

AF = mybir.ActivationFunctionType
OP = mybir.AluOpType


def _emit(nc, tc):
    from contextlib import ExitStack

    with ExitStack() as ctx:
        big = ctx.enter_context(tc.tile_pool(name="big", bufs=26))
        wqp = ctx.enter_context(tc.tile_pool(name="wqp", bufs=NCT))
        pwp = ctx.enter_context(tc.tile_pool(name="pwp", bufs=NCT))
        ckp = ctx.enter_context(tc.tile_pool(name="ckp", bufs=NCT))
        dwp = ctx.enter_context(tc.tile_pool(name="dwp", bufs=NCT))
        dgp = ctx.enter_context(tc.tile_pool(name="dgp", bufs=PE_CONV_CT * K + 1))
        kxp = ctx.enter_context(tc.tile_pool(name="kxp", bufs=2))
        onp = ctx.enter_context(tc.tile_pool(name="onp", bufs=1))
        on2 = ctx.enter_context(tc.tile_pool(name="on2", bufs=1))
        psp = ctx.enter_context(tc.tile_pool(name="psp", bufs=8, space="PSUM"))

        xT_d = nc.dram_tensor("xT", [C, L], BF16, kind="ExternalInput")
        wqco_d = nc.dram_tensor("wqco", [C, 2 * C], BF16, kind="ExternalInput")
        pwT_d = nc.dram_tensor("pwT", [C, C], BF16, kind="ExternalInput")
        wck_d = nc.dram_tensor("wck", [C, HK], BF16, kind="ExternalInput")
        dws_d = nc.dram_tensor("dws", [C, K], F32, kind="ExternalInput")
        bqco_d = nc.dram_tensor("bqco", [P, 2 * NCT], F32, kind="ExternalInput")
        bsep_d = nc.dram_tensor("bsep", [P, NCT], F32, kind="ExternalInput")
        bck_d = nc.dram_tensor("bck", [HK, 1], F32, kind="ExternalInput")
        out_d = nc.dram_tensor("out", [C, L], BF16, kind="ExternalOutput")
        expT_dram = nc.dram_tensor("expTd", [HK, L], BF16)
        recipT_dram = nc.dram_tensor("recipTd", [H, L], BF16)

        # ---- weights / constants ----
        wqco = [wqp.tile([P, 2 * C], BF16, tag="wq", name=f"wqco{i}") for i in range(NCT)]
        pwT = [pwp.tile([P, C], BF16, tag="pw", name=f"pwT{i}") for i in range(NCT)]
        wck = [ckp.tile([P, HK], BF16, tag="ck", name=f"wck{i}") for i in range(NCT)]
        dws = [dwp.tile([P, K], F32, tag="dw", name=f"dws{i}") for i in range(NCT)]
        for g in range(NCT):
            sl = slice(g * P, (g + 1) * P)
            nc.sync.dma_start(wqco[g][:], wqco_d[sl, :])
            nc.sync.dma_start(pwT[g][:], pwT_d[sl, :])
            nc.sync.dma_start(wck[g][:], wck_d[sl, :])
            nc.sync.dma_start(dws[g][:], dws_d[sl, :])
        bqco = onp.tile([P, 2 * NCT], F32, tag="bq")
        bsep = onp.tile([P, NCT], F32, tag="bs")
        bck = onp.tile([HK, 1], F32, tag="bk")
        nc.sync.dma_start(bqco[:], bqco_d[:])
        nc.sync.dma_start(bsep[:], bsep_d[:])
        nc.sync.dma_start(bck[:], bck_d[:])

        ident = dgp.tile([P, P], BF16, tag="dg")
        make_identity(nc, ident[:])
        # ones_block[p, h] = 1 iff p // 9 == h  (for summing exp over k)
        ones = on2.tile([HK, H], BF16, tag="on")
        nc.gpsimd.memset(ones[:], 1.0)
        nc.gpsimd.affine_select(
            out=ones[:], in_=ones[:], compare_op=OP.is_ge, fill=0.0,
            base=0, pattern=[[-K, H]], channel_multiplier=1)
        nc.gpsimd.affine_select(
            out=ones[:], in_=ones[:], compare_op=OP.is_ge, fill=0.0,
            base=K - 1, pattern=[[K, H]], channel_multiplier=-1)

        diag = {}
        for g in range(PE_CONV_CT):
            for k in range(K):
                d = dgp.tile([P, P], BF16, tag="dg", name=f"diag{g}_{k}")
                nc.vector.tensor_scalar_mul(d[:], ident[:], dws[g][:, k : k + 1])
                diag[(g, k)] = d

        # ---- x load (padded for conv halo) ----
        xT = []
        for g in range(NCT):
            t = big.tile([P, L + 2 * PAD], BF16, tag="big", name=f"xT{g}")
            nc.gpsimd.memset(t[:, 0:PAD], 0.0)
            nc.gpsimd.memset(t[:, L + PAD : L + 2 * PAD], 0.0)
            nc.sync.dma_start(t[:, PAD : L + PAD], xT_d[g * P : (g + 1) * P, :])
            xT.append(t)

        # ---- depthwise conv -> dwout [C, L] ----
        dwout = []
        for g in range(NCT):
            t = big.tile([P, L], BF16, tag="big", name=f"dwout{g}")
            dwout.append(t)
        for g in range(PE_CONV_CT):  # TensorE: 9 accumulating diag matmuls
            pss = [psp.tile([P, LC], F32, tag="ps", name="ps") for _ in range(NLC)]
            for k in range(K):
                for oc in range(NLC):
                    nc.tensor.matmul(
                        pss[oc][:], diag[(g, k)][:],
                        xT[g][:, oc * LC + k : oc * LC + k + LC],
                        start=(k == 0), stop=(k == K - 1))
            for oc in range(NLC):
                nc.scalar.copy(dwout[g][:, oc * LC : (oc + 1) * LC], pss[oc][:])
        for g in range(PE_CONV_CT, NCT):  # DVE: per-partition-scalar MAC chain
            nc.vector.tensor_scalar_mul(dwout[g][:], xT[g][:, 0:L], dws[g][:, 0:1])
            for k in range(1, K):
                nc.vector.scalar_tensor_tensor(
                    out=dwout[g][:], in0=xT[g][:, k : k + L],
                    scalar=dws[g][:, k : k + 1], in1=dwout[g][:],
                    op0=OP.mult, op1=OP.add)

        # ---- q | co projections (fused): out[o, l] = sum_c W[c, o] * xT[c, l] ----
        q = [big.tile([P, L], BF16, tag="big", name=f"q{i}") for i in range(NCT)]
        co = []
        for g in range(NCT):
            t = big.tile([P, L + 2 * PAD], BF16, tag="big", name=f"co{g}")
            nc.gpsimd.memset(t[:, 0:PAD], 0.0)
            nc.gpsimd.memset(t[:, L + PAD : L + 2 * PAD], 0.0)
            co.append(t)
        for ot in range(2 * NCT):
            pss = [psp.tile([P, LC], F32, tag="ps", name="ps") for _ in range(NLC)]
            for g in range(NCT):
                for oc in range(NLC):
                    nc.tensor.matmul(
                        pss[oc][:], wqco[g][:, ot * P : (ot + 1) * P],
                        xT[g][:, PAD + oc * LC : PAD + (oc + 1) * LC],
                        start=(g == 0), stop=(g == NCT - 1))
            for oc in range(NLC):
                if ot < NCT:
                    dst = q[ot][:, oc * LC : (oc + 1) * LC]
                else:
                    dst = co[ot - NCT][:, PAD + oc * LC : PAD + (oc + 1) * LC]
                nc.scalar.activation(dst, pss[oc][:], AF.Identity,
                                     bias=bqco[:, ot : ot + 1])

        # ---- key = pw^T @ dwout ----
        key = [big.tile([P, L], BF16, tag="big", name=f"key{i}") for i in range(NCT)]
        for ot in range(NCT):
            pss = [psp.tile([P, LC], F32, tag="ps", name="ps") for _ in range(NLC)]
            for g in range(NCT):
                for oc in range(NLC):
                    nc.tensor.matmul(
                        pss[oc][:], pwT[g][:, ot * P : (ot + 1) * P],
                        dwout[g][:, oc * LC : (oc + 1) * LC],
                        start=(g == 0), stop=(g == NCT - 1))
            for oc in range(NLC):
                nc.scalar.activation(key[ot][:, oc * LC : (oc + 1) * LC], pss[oc][:],
                                     AF.Identity, bias=bsep[:, ot : ot + 1])

        # ---- attn = key * q (in-place into key) ----
        for g in range(NCT):
            nc.vector.tensor_mul(key[g][:], key[g][:], q[g][:])

        # ---- per-chunk softmax tail: logits -> exp -> sums -> recip ->
        # ---- 9-fold recip broadcast -> normalize -> DRAM stage (pipelined) ----
        expT = onp.tile([HK, L], BF16, tag="ex")
        recipT = onp.tile([H, L], BF16, tag="rc")
        recip9 = onp.tile([HK, L], BF16, tag="r9")
        rb = recipT_dram[:]
        for oc in range(NLC):
            sl = slice(oc * LC, (oc + 1) * LC)
            ps = psp.tile([HK, LC], F32, tag="ps", name="psk")
            for g in range(NCT):
                nc.tensor.matmul(
                    ps[:], wck[g][:], key[g][:, sl],
                    start=(g == 0), stop=(g == NCT - 1))
            nc.scalar.activation(expT[:, sl], ps[:], AF.Exp, bias=bck[:, 0:1])
            ps2 = psp.tile([H, LC], F32, tag="ps", name="pss")
            nc.tensor.matmul(ps2[:], ones[:], expT[:, sl], start=True, stop=True)
            with nc.allow_low_precision(reason="bf16 softmax denominators"):
                nc.vector.reciprocal(recipT[:, sl], ps2[:])
            nc.sync.dma_start(recipT_dram[:, sl], recipT[:, sl])
            nc.sync.dma_start(
                recip9[:, sl],
                bass.AP(rb.tensor, oc * LC, [[L, H], [0, K], [1, LC]]))
            nc.vector.tensor_mul(expT[:, sl], expT[:, sl], recip9[:, sl])
            nc.sync.dma_start(expT_dram[:, sl], expT[:, sl])

        # ---- dynamic conv einsum + normalization ----
        out_cl = [big.tile([P, L], BF16, tag="big", name=f"outcl{i}") for i in range(NCT)]
        LH = L // 2
        for g in range(NCT):
            for oc in range(2):
                kx = kxp.tile([P, K, LH], BF16, tag="kx", name=f"kx{g}_{oc}")
                eb = expT_dram[:]
                for hh in range(2):
                    sap = bass.AP(eb.tensor,
                                  K * (2 * g + hh) * L + oc * LH,
                                  [[0, 64], [L, K], [1, LH]])
                    nc.sync.dma_start(kx[hh * 64 : (hh + 1) * 64, :, :], sap)

                base = co[g][:]
                win = bass.AP(base.tensor, base.offset + oc * LH,
                              [list(base.ap)[0], [1, K], [1, LH]])
                nc.vector.tensor_mul(kx[:], win, kx[:])
                nc.vector.tensor_add(kx[:, 0:4, :], kx[:, 0:4, :], kx[:, 4:8, :])
                nc.vector.tensor_add(kx[:, 0:2, :], kx[:, 0:2, :], kx[:, 2:4, :])
                nc.vector.tensor_add(kx[:, 0, :], kx[:, 0, :], kx[:, 1, :])
                nc.vector.tensor_add(out_cl[g][:, oc * LH : (oc + 1) * LH],
                                     kx[:, 0, :], kx[:, 8, :])

        for g in range(NCT):
            nc.sync.dma_start(out_d[g * P : (g + 1) * P, :], out_cl[g][:])


_NC_CACHE = None


def _build():
    global _NC_CACHE
    if _NC_CACHE is None:
        nc = bacc.Bacc("TRN2", target_bir_lowering=False, debug=False)
        with tile.TileContext(nc) as tc:
            _emit(nc, tc)
        nc.compile()
        _NC_CACHE = nc
    return _NC_CACHE


def _host_inputs(hidden_states, W_q, dw, pw, W_ck, W_co, b_q, b_co, sep_bias, b_ck):
    bf = ml_dtypes.bfloat16
    wqco = np.concatenate([W_q, W_co], axis=1).astype(bf)
    pwT = np.ascontiguousarray(pw.T).astype(bf)
    wck = W_ck.astype(bf)
    dws = np.asarray(dw, np.float32).reshape(C, K)
    bqco = np.concatenate([b_q.reshape(NCT, P), b_co.reshape(NCT, P)], axis=0)
    bqco = np.ascontiguousarray(bqco.T).astype(np.float32)  # [128, 12], col = otile
    bsep = np.ascontiguousarray(sep_bias.reshape(NCT, P).T).astype(np.float32)
    bck = np.asarray(b_ck, np.float32).reshape(HK, 1)
    shared = {"wqco": wqco, "pwT": pwT, "wck": wck, "dws": dws,
              "bqco": bqco, "bsep": bsep, "bck": bck}
    maps = []
    for b in range(B):
        xT = np.ascontiguousarray(np.asarray(hidden_states[b]).T).astype(bf)
        m = dict(shared)
        m["xT"] = xT
        maps.append(m)
    return maps


def kernel(hidden_states, W_q, b_q, dw, pw, sep_bias, W_ck, b_ck, W_co, b_co):
    hidden_states = np.asarray(hidden_states, np.float32)
    nc = _build()
    maps = _host_inputs(hidden_states, np.asarray(W_q, np.float32),
                        np.asarray(dw, np.float32), np.asarray(pw, np.float32),
                        np.asarray(W_ck, np.float32), np.asarray(W_co, np.float32),
                        np.asarray(b_q, np.float32), np.asarray(b_co, np.float32),
                        np.asarray(sep_bias, np.float32), np.asarray(b_ck, np.float32))
    res = run_bass_kernel_spmd(nc, maps, list(range(B)))
    out = np.empty((B, L, C), np.float32)
    for b in range(B):
        out[b] = np.asarray(res.results[b]["out"]).T.astype(np.float32)
    return out



# revision 10
# speedup vs baseline: 1.4282x; 1.4282x over previous
"""ConvBERT attention block (SeparableConv1D key + dynamic conv) on 8 TRN2 NeuronCores.

Sharding: data-parallel over batch (B=8 -> 1 sample per core), weights replicated.

Per-core dataflow ([C, L] layout, channels on partitions), software-pipelined over
four 512-column l-chunks so PE / ACT / DVE / Pool / DMA overlap across chunks:

  stage A (chunk a):  q = Wq^T x   (fp8 DoubleRow, 2x PE)
                      co = Wco^T x (bf16)
                      dwout = depthwise-conv x (plain-fp8 diag matmuls on PE)
                      key = pw^T dwout (fp8 DoubleRow)
                      attn = key * q (DVE)
  stage B (chunk a-1): logits = Wck^T attn (bf16) -> exp on ACT -> sums (ones
                      matmul) -> recip (DVE) -> 9-fold recip bcast via DRAM ->
                      expT normalized, staged to DRAM -> kx 64-way bcast DMAs
  stage C (chunk a-2): einsum out[c,l] = sum_k co[c,l+k-4]*kx[hk,l]
                      (windowed mult + tree adds, split DVE / Pool) -> out DMA

fp8 only feeds the q/key/logits path; its error is crushed by the softmax
(logits are tiny), leaving final rel-err ~5e-3. co stays bf16.
"""

import os
import sys

for _p in ("/opt/trn_rl_repo", "/root/.axon_site/_ro/trn_rl_repo"):
    if os.path.isdir(_p) and _p not in sys.path:
        sys.path.append(_p)

import ml_dtypes
import numpy as np

import concourse.bass as bass
import concourse.mybir as mybir
import concourse.tile as tile
from concourse import bacc
from concourse.bass_utils import run_bass_kernel_spmd

BF16 = mybir.dt.bfloat16
F32 = mybir.dt.float32
FP8 = mybir.dt.float8e4

H, D, K = 12, 64, 9
C = H * D  # 768
L = 2048
B = 8
PAD = (K - 1) // 2  # 4
P = 128
NCT = C // P  # 6 channel tiles
LC = 512
NLC = L // LC  # 4
HK = H * K  # 108
XROW = L + 16  # x8 row pitch: left pad 4, right pad 12 (keeps DR plane stride %16==0)
CROW = L + 8  # co row pitch (pad 4 both sides)

SW = 64.0  # fp8 weight scale (Wq, pw, dw)
SD = 32.0  # fp8 dwout scale

AF = mybir.ActivationFunctionType
OP = mybir.AluOpType
DR = mybir.MatmulPerfMode.DoubleRow

# einsum units routed to the Pool (gpsimd) engine instead of DVE
POOL_UNITS = {(0, 0), (0, 1), (0, 2), (0, 3), (1, 1)}




def _emit(nc, tc):
    from contextlib import ExitStack

    with ExitStack() as ctx:
        prs = ctx.enter_context(tc.tile_pool(name="prs", bufs=1))
        wcop = ctx.enter_context(tc.tile_pool(name="wcop", bufs=NCT))
        wckp = ctx.enter_context(tc.tile_pool(name="wckp", bufs=NCT))
        cop = ctx.enter_context(tc.tile_pool(name="cop", bufs=NCT))
        xtp = ctx.enter_context(tc.tile_pool(name="xtp", bufs=12))
        qp = ctx.enter_context(tc.tile_pool(name="qp", bufs=12))
        kp = ctx.enter_context(tc.tile_pool(name="kp", bufs=12))
        r9p = ctx.enter_context(tc.tile_pool(name="r9p", bufs=2))
        kxp = ctx.enter_context(tc.tile_pool(name="kxp", bufs=7))
        outp = ctx.enter_context(tc.tile_pool(name="outp", bufs=12))
        psb = ctx.enter_context(tc.tile_pool(name="psb", bufs=6, space="PSUM"))
        psl = ctx.enter_context(tc.tile_pool(name="psl", bufs=1, space="PSUM"))
        pss = ctx.enter_context(tc.tile_pool(name="pss", bufs=1, space="PSUM"))

        xT_d = nc.dram_tensor("xT", [C, L], BF16, kind="ExternalInput")
        x8_d = nc.dram_tensor("x8", [C, L], FP8, kind="ExternalInput")
        wq8_d = nc.dram_tensor("wq8", [P, 6 * C], FP8, kind="ExternalInput")
        pw8_d = nc.dram_tensor("pw8", [P, 6 * C], FP8, kind="ExternalInput")
        dg8_d = nc.dram_tensor("dg8", [P, NCT * K * P], FP8, kind="ExternalInput")
        wco_d = nc.dram_tensor("wco", [C, C], BF16, kind="ExternalInput")
        wck_d = nc.dram_tensor("wck", [C, HK], BF16, kind="ExternalInput")
        bq_d = nc.dram_tensor("bq", [P, NCT], F32, kind="ExternalInput")
        bco_d = nc.dram_tensor("bco", [P, NCT], F32, kind="ExternalInput")
        bsep_d = nc.dram_tensor("bsep", [P, NCT], F32, kind="ExternalInput")
        bck_d = nc.dram_tensor("bck", [HK, 1], F32, kind="ExternalInput")
        out_d = nc.dram_tensor("out", [C, L], BF16, kind="ExternalOutput")
        expT_dram = nc.dram_tensor("expTd", [HK, L], BF16)
        recipT_dram = nc.dram_tensor("recipTd", [H, L], BF16)

        # ---- persistent weights / constants ----
        wq8 = prs.tile([P, 6 * C], FP8, tag="wq8", name="wq8")
        pw8 = prs.tile([P, 6 * C], FP8, tag="pw8", name="pw8")
        dg8 = prs.tile([P, NCT * K * P], FP8, tag="dg8", name="dg8")
        nc.sync.dma_start(wq8[:], wq8_d[:])
        nc.sync.dma_start(pw8[:], pw8_d[:])
        nc.sync.dma_start(dg8[:], dg8_d[:])
        wco = [wcop.tile([P, C], BF16, tag="wco", name=f"wco{g}") for g in range(NCT)]
        wck = [wckp.tile([P, HK], BF16, tag="wck", name=f"wck{g}") for g in range(NCT)]
        for g in range(NCT):
            sl = slice(g * P, (g + 1) * P)
            nc.sync.dma_start(wco[g][:], wco_d[sl, :])
            nc.sync.dma_start(wck[g][:], wck_d[sl, :])
        bq = prs.tile([P, NCT], F32, tag="bq", name="bq")
        bco = prs.tile([P, NCT], F32, tag="bco", name="bco")
        bsep = prs.tile([P, NCT], F32, tag="bsep", name="bsep")
        bck = prs.tile([HK, 1], F32, tag="bck", name="bck")
        nc.sync.dma_start(bq[:], bq_d[:])
        nc.sync.dma_start(bco[:], bco_d[:])
        nc.sync.dma_start(bsep[:], bsep_d[:])
        nc.sync.dma_start(bck[:], bck_d[:])
        # ones_block[hk, h] = 1 iff hk // 9 == h (sums exp over k)
        ones = prs.tile([HK, H], BF16, tag="ones", name="ones")
        nc.gpsimd.memset(ones[:], 1.0)
        nc.gpsimd.affine_select(
            out=ones[:], in_=ones[:], compare_op=OP.is_ge, fill=0.0,
            base=0, pattern=[[-K, H]], channel_multiplier=1)
        nc.gpsimd.affine_select(
            out=ones[:], in_=ones[:], compare_op=OP.is_ge, fill=0.0,
            base=K - 1, pattern=[[K, H]], channel_multiplier=-1)

        # ---- persistent activations ----
        x8 = prs.tile([P, NCT * XROW], FP8, tag="x8", name="x8")  # x, padded rows
        dw8 = prs.tile([P, NCT * L], FP8, tag="dw8", name="dw8")  # dwout * SD
        co = [cop.tile([P, CROW], BF16, tag="co", name=f"co{g}") for g in range(NCT)]
        expT = prs.tile([HK, L], BF16, tag="expT", name="expT")
        recipT = prs.tile([H, L], BF16, tag="recipT", name="recipT")
        for g in range(NCT):
            nc.gpsimd.memset(
                bass.AP(x8.tensor, x8.offset + g * XROW, [[NCT * XROW, P], [1, PAD]]), 0.0)
            nc.gpsimd.memset(
                bass.AP(x8.tensor, x8.offset + g * XROW + PAD + L,
                        [[NCT * XROW, P], [1, XROW - PAD - L]]), 0.0)
            nc.gpsimd.memset(co[g][:, 0:PAD], 0.0)
            nc.gpsimd.memset(co[g][:, PAD + L:CROW], 0.0)

        xap = x8.ap[0]  # [NCT*XROW, P]

        def x8ap(g, off, dims):
            return bass.AP(x8.tensor, x8.offset + g * XROW + off, [list(xap)] + dims)

        attn = {}  # (ot, chunk) -> tile (q tile, multiplied in place)

        # ================= pipeline =================
        for s in range(NLC + 3):
            # ---- x loads for chunk s ----
            if s < NLC:
                cs = slice(s * LC, (s + 1) * LC)
                for g in range(NCT):
                    nc.sync.dma_start(
                        x8ap(g, PAD + s * LC, [[1, LC]]), x8_d[g * P:(g + 1) * P, cs])
                    t = xtp.tile([P, LC], BF16, tag="xt", name=f"xt{g}_{s}")
                    nc.sync.dma_start(t[:], xT_d[g * P:(g + 1) * P, cs])
                    attn[("x", g, s)] = t

            # ---- stage A: projections + conv + attn for chunk a ----
            a = s - 1
            if 0 <= a < NLC:
                asl = slice(a * LC, (a + 1) * LC)
                # q (fp8 DoubleRow)
                for ot in range(NCT):
                    ps = psb.tile([P, LC], F32, tag="ps", name="psq")
                    for j in range(3):
                        lhsT = bass.AP(wq8.tensor, wq8.offset + j * 2 * C + ot * P,
                                       [list(wq8.ap[0]), [C, 2], [1, P]])
                        rhs = x8ap(2 * j, PAD + a * LC, [[XROW, 2], [1, LC]])
                        nc.tensor.matmul(ps[:], lhsT, rhs, start=(j == 0),
                                         stop=(j == 2), perf_mode=DR)
                    qt = qp.tile([P, LC], BF16, tag="q", name=f"q{ot}_{a}")
                    nc.scalar.activation(qt[:], ps[:], AF.Identity,
                                         bias=bq[:, ot:ot + 1], scale=1.0 / SW)
                    attn[(ot, a)] = qt
                # co (bf16)
                for ot in range(NCT):
                    ps = psb.tile([P, LC], F32, tag="ps", name="psc")
                    for g in range(NCT):
                        nc.tensor.matmul(
                            ps[:], wco[g][:, ot * P:(ot + 1) * P],
                            attn[("x", g, a)][:], start=(g == 0), stop=(g == NCT - 1))
                    nc.scalar.activation(co[ot][:, PAD + a * LC:PAD + (a + 1) * LC],
                                         ps[:], AF.Identity, bias=bco[:, ot:ot + 1])
                # depthwise conv (plain fp8 diag matmuls)
                for g in range(NCT):
                    ps = psb.tile([P, LC], F32, tag="ps", name="psd")
                    for k in range(K):
                        lhsT = bass.AP(dg8.tensor, dg8.offset + (g * K + k) * P,
                                       [list(dg8.ap[0]), [1, P]])
                        rhs = x8ap(g, a * LC + k, [[1, LC]])
                        nc.tensor.matmul(ps[:], lhsT, rhs, start=(k == 0),
                                         stop=(k == K - 1))
                    nc.scalar.activation(
                        bass.AP(dw8.tensor, dw8.offset + g * L + a * LC,
                                [list(dw8.ap[0]), [1, LC]]),
                        ps[:], AF.Copy, scale=SD / SW)
                # key (fp8 DoubleRow) + attn mult
                for ot in range(NCT):
                    ps = psb.tile([P, LC], F32, tag="ps", name="psk")
                    for j in range(3):
                        lhsT = bass.AP(pw8.tensor, pw8.offset + j * 2 * C + ot * P,
                                       [list(pw8.ap[0]), [C, 2], [1, P]])
                        rhs = bass.AP(dw8.tensor, dw8.offset + 2 * j * L + a * LC,
                                      [list(dw8.ap[0]), [L, 2], [1, LC]])
                        nc.tensor.matmul(ps[:], lhsT, rhs, start=(j == 0),
                                         stop=(j == 2), perf_mode=DR)
                    kt = kp.tile([P, LC], BF16, tag="k", name=f"k{ot}_{a}")
                    nc.scalar.activation(kt[:], ps[:], AF.Identity,
                                         bias=bsep[:, ot:ot + 1], scale=1.0 / (SW * SD))
                    nc.vector.tensor_mul(attn[(ot, a)][:], attn[(ot, a)][:], kt[:])

            # ---- stage B: softmax + kx broadcast for chunk b ----
            b = s - 2
            if 0 <= b < NLC:
                bsl = slice(b * LC, (b + 1) * LC)
                ps = psl.tile([HK, LC], F32, tag="psl", name="pslg")
                for g in range(NCT):
                    nc.tensor.matmul(ps[:], wck[g][:], attn[(g, b)][:],
                                     start=(g == 0), stop=(g == NCT - 1))
                nc.scalar.activation(expT[:, bsl], ps[:], AF.Exp, bias=bck[:, 0:1])
                ps2 = pss.tile([H, LC], F32, tag="pss", name="psss")
                nc.tensor.matmul(ps2[:], ones[:], expT[:, bsl], start=True, stop=True)
                with nc.allow_low_precision(reason="bf16 softmax denominators"):
                    nc.vector.reciprocal(recipT[:, bsl], ps2[:])
                nc.scalar.dma_start(recipT_dram[:, bsl], recipT[:, bsl])
                r9 = r9p.tile([HK, LC], BF16, tag="r9", name=f"r9_{b}")
                rb = recipT_dram[:]
                nc.scalar.dma_start(
                    r9[:], bass.AP(rb.tensor, b * LC, [[L, H], [0, K], [1, LC]]))
                nc.vector.tensor_mul(expT[:, bsl], expT[:, bsl], r9[:])
                nc.scalar.dma_start(expT_dram[:, bsl], expT[:, bsl])
                eb = expT_dram[:]
                for g in range(NCT):
                    kx = kxp.tile([P, K, LC], BF16, tag="kx", name=f"kx{g}_{b}")
                    for hh in range(2):
                        sap = bass.AP(eb.tensor, K * (2 * g + hh) * L + b * LC,
                                      [[0, 64], [L, K], [1, LC]])
                        nc.sync.dma_start(kx[hh * 64:(hh + 1) * 64, :, :], sap)
                    attn[("kx", g, b)] = kx

            # ---- stage C: dynamic-conv einsum for chunk c ----
            c = s - 3
            if 0 <= c < NLC:
                for g in range(NCT):
                    kx = attn.pop(("kx", g, c))
                    base = co[g][:]
                    win = bass.AP(base.tensor, base.offset + c * LC,
                                  [list(base.ap)[0], [1, K], [1, LC]])
                    ot = outp.tile([P, LC], BF16, tag="o", name=f"o{g}_{c}")
                    e = nc.gpsimd if (g, c) in POOL_UNITS else nc.vector

                    def tt(out, i0, i1, op1):
                        if op1 == OP.mult:
                            e.tensor_mul(out, i0, i1)
                        else:
                            e.tensor_add(out, i0, i1)
                    tt(kx[:], win, kx[:], OP.mult)
                    tt(kx[:, 0:4, :], kx[:, 0:4, :], kx[:, 4:8, :], OP.add)
                    tt(kx[:, 0:2, :], kx[:, 0:2, :], kx[:, 2:4, :], OP.add)
                    tt(kx[:, 0, :], kx[:, 0, :], kx[:, 1, :], OP.add)
                    tt(ot[:], kx[:, 0, :], kx[:, 8, :], OP.add)
                    nc.sync.dma_start(
                        out_d[g * P:(g + 1) * P, c * LC:(c + 1) * LC], ot[:])


_NC_CACHE = None


def _build():
    global _NC_CACHE
    if _NC_CACHE is None:
        nc = bacc.Bacc("TRN2", target_bir_lowering=False, debug=False)
        with tile.TileContext(nc) as tc:
            _emit(nc, tc)
        nc.compile()
        _NC_CACHE = nc
    return _NC_CACHE


def _host_inputs(hidden_states, W_q, dw, pw, W_ck, W_co, b_q, b_co, sep_bias, b_ck):
    bf = ml_dtypes.bfloat16
    f8 = ml_dtypes.float8_e4m3
    wq8 = np.ascontiguousarray(
        (W_q * SW).reshape(3, 2, P, C).transpose(2, 0, 1, 3).reshape(P, 6 * C)
    ).astype(f8)
    pw8 = np.ascontiguousarray(
        (pw.T * SW).reshape(3, 2, P, C).transpose(2, 0, 1, 3).reshape(P, 6 * C)
    ).astype(f8)
    dws = np.asarray(dw, np.float32).reshape(C, K)
    dg = np.zeros((P, NCT, K, P), np.float32)
    idx = np.arange(P)
    for g in range(NCT):
        for k in range(K):
            dg[idx, g, k, idx] = dws[g * P + idx, k] * SW
    dg8 = np.ascontiguousarray(dg.reshape(P, NCT * K * P)).astype(f8)
    wco = W_co.astype(bf)
    wck = W_ck.astype(bf)
    bq = np.ascontiguousarray(b_q.reshape(NCT, P).T).astype(np.float32)
    bcoh = np.ascontiguousarray(b_co.reshape(NCT, P).T).astype(np.float32)
    bsep = np.ascontiguousarray(sep_bias.reshape(NCT, P).T).astype(np.float32)
    bck = np.asarray(b_ck, np.float32).reshape(HK, 1)
    shared = {"wq8": wq8, "pw8": pw8, "dg8": dg8, "wco": wco, "wck": wck,
              "bq": bq, "bco": bcoh, "bsep": bsep, "bck": bck}
    maps = []
    for b in range(B):
        xT = np.ascontiguousarray(np.asarray(hidden_states[b]).T)
        m = dict(shared)
        m["xT"] = xT.astype(bf)
        m["x8"] = xT.astype(f8)
        maps.append(m)
    return maps


def kernel(hidden_states, W_q, b_q, dw, pw, sep_bias, W_ck, b_ck, W_co, b_co):
    hidden_states = np.asarray(hidden_states, np.float32)
    nc = _build()
    maps = _host_inputs(hidden_states, np.asarray(W_q, np.float32),
                        np.asarray(dw, np.float32), np.asarray(pw, np.float32),
                        np.asarray(W_ck, np.float32), np.asarray(W_co, np.float32),
                        np.asarray(b_q, np.float32), np.asarray(b_co, np.float32),
                        np.asarray(sep_bias, np.float32), np.asarray(b_ck, np.float32))
    res = run_bass_kernel_spmd(nc, maps, list(range(B)))
    out = np.empty((B, L, C), np.float32)
    for b in range(B):
        out[b] = np.asarray(res.results[b]["out"]).T.astype(np.float32)
    return out
